# revision 8
# baseline (speedup 1.0000x reference)
"""Trainium2 Bass kernel for scatter_memory problem nn_Memory_value_57475252355404.

out[b, dispatch[b,e,c], :] += weight[indices[b,e,c], :] * score[b,e,c]

Strategy (8 cores, SPMD single program, ONE launch):
  - Shard the TABLE row-wise: core k owns rows [k*32768, (k+1)*32768) and
    receives ONLY that 8MB bf16 slice as its per-core "weight" input, so
    the single SPMD program always gathers from window [0, 32768) and an
    int16 idx covers it exactly. Tokens are routed to cores by idx>>15.
  - Gather via SWDGE dma_gather (mlp ucode), ONE call per SWDGE queue
    (4 calls, descending sizes): no second wave per queue, so no
    ring-drain blocking. num_idxs is a compile-time constant (no cnt
    registers), pad slots point at row 0 with score 0 (no memset).
  - A dummy 128-idx gather from a memset idx tile is issued first so the
    one-time ~6us ucode IRAM load overlaps the input DMAs.
  - Scatter-add via per-block one-hot bf16 matmuls: block g = 128
    dest-sorted tokens; each distinct dest row in a block gets a rank
    slot; onehot[t, g*128+r] = (iota[r] == destrel[t,g]) * score[t,g],
    built with TWO whole-tile DVE tensor_tensor ops using broadcast APs;
    the PE computes psum[d, r] = sum_t tok[t, d] * onehot[t, r], 4 groups
    per PSUM bank; ACT copies each bank to a bf16 buffer; out-DMA per
    3 banks.
  - Host: rank slots -> physical rows (np.add.at in f32) over the full
    [B*N, D] output (cores may hit any row).
"""

import sys

sys.path.insert(0, "/opt/trn_rl_repo")

import numpy as np
import ml_dtypes

BF16 = ml_dtypes.bfloat16

B, E, C = 4, 16, 512
EC = E * C
V, D = 262144, 128
N = 4096
NCORES = 8
WIN = V // NCORES  # 32768 rows per core window
NQ = 4  # SWDGE queues / gather calls

_cache = {}
LAST_RESULTS = None  # BassKernelResults of the most recent run (for test.py)


MAXG_CALL = 8  # SWDGE ring holds 1024 descriptors -> at most 8*128 idxs/call


def _plan_calls(G):
    """Split G groups into per-queue gather calls of <=MAXG_CALL groups.

    Returns list of (queue, g_start, g_len) in EMISSION order: all wave-1
    calls first (one per queue), then wave-2 remainders. A wave-2 call's
    desc-gen blocks the Q7 engine until its queue's wave-1 call drains, so
    wave-2 must come after every wave-1 gen.
    """
    base, rem = divmod(G, NQ)
    loads = [base + (1 if q < rem else 0) for q in range(NQ)]
    assert all(ld <= 2 * MAXG_CALL for ld in loads), loads
    starts = np.cumsum([0] + loads).tolist()
    wave1, wave2 = [], []
    for q in range(NQ):
        g0, ld = starts[q], loads[q]
        c1 = min(MAXG_CALL, ld)
        wave1.append((q, g0, c1))
        if ld > c1:
            wave2.append((q, g0 + c1, ld - c1))
    return wave1 + wave2


def _build(G, dummy=True, bigtt=True):
    from concourse import bacc, tile, mybir, library_config

    f32 = mybir.dt.float32
    bf16 = mybir.dt.bfloat16
    i16 = mybir.dt.int16

    TOT = G * 128
    calls = _plan_calls(G)

    nc = bacc.Bacc(
        "TRN2",
        target_bir_lowering=False,
        debug=False,
        num_devices=NCORES,
        num_swdge_queues=NQ,
    )
    w = nc.dram_tensor("weight", [WIN, D], bf16, kind="ExternalInput")
    gi = nc.dram_tensor("gidx", [128, TOT // 16], i16, kind="ExternalInput")
    # meta = score_s [128, G] | destrel [128, G] | iota [128, 128], all bf16
    meta = nc.dram_tensor("meta", [128, 2 * G + 128], bf16, kind="ExternalInput")
    out = nc.dram_tensor("out", [128, TOT], bf16, kind="ExternalOutput")

    with tile.TileContext(nc) as tc:
        with tc.tile_pool(name="p", bufs=1) as pool, \
             tc.tile_pool(name="ps", bufs=8, space="PSUM") as psp:
            # start the gpsimd ucode library load immediately; the dummy
            # gather right after pays the one-time IRAM load (~6us) while
            # the input DMAs are still in flight
            nc.gpsimd.load_library(library_config.mlp)
            wap = w.ap()
            if dummy:
                dum_i = pool.tile([128, 8], i16)
                nc.vector.memset(dum_i[:], 0)
                dum_o = pool.tile([128, 1, D], bf16)
                nc.gpsimd.dma_gather(
                    dum_o[:], wap, dum_i[:], 128, 128, D, queue_num=0
                )

            gi_t = pool.tile([128, TOT // 16], i16)
            nc.sync.dma_start(gi_t[:], gi.ap())
            meta_t = pool.tile([128, 2 * G + 128], bf16)
            nc.sync.dma_start(meta_t[:], meta.ap())

            tok = pool.tile([128, G, D], bf16)
            oh = pool.tile([128, G, 128], bf16)
            osb = pool.tile([128, TOT], bf16)

            # wave-1 calls (one per queue) then small wave-2 remainders
            for q, g0, glen in calls:
                cap = glen * 128
                off = g0 * 128
                nc.gpsimd.dma_gather(
                    tok[:, g0 : g0 + glen, :],
                    wap,
                    gi_t[:, off // 16 : (off + cap) // 16],
                    cap,
                    cap,
                    D,
                    queue_num=q,
                )

            # onehot[t, g, r] = (iota[r] == destrel[t,g]) * score[t,g]
            if bigtt:
                io_bc = meta_t[:, 2 * G : 2 * G + 128][:, None, :].to_broadcast(
                    [128, G, 128]
                )
                dr_bc = meta_t[:, G : 2 * G, None].to_broadcast([128, G, 128])
                sc_bc = meta_t[:, 0:G, None].to_broadcast([128, G, 128])
                nc.vector.tensor_tensor(oh[:], io_bc, dr_bc, mybir.AluOpType.is_equal)
                nc.vector.tensor_tensor(oh[:], oh[:], sc_bc, mybir.AluOpType.mult)
            else:
                io_t = meta_t[:, 2 * G : 2 * G + 128]
                for g in range(G):
                    nc.vector.tensor_tensor(
                        oh[:, g, :],
                        io_t,
                        meta_t[:, G + g : G + g + 1].to_broadcast([128, 128]),
                        mybir.AluOpType.is_equal,
                    )
                    nc.vector.tensor_tensor(
                        oh[:, g, :],
                        oh[:, g, :],
                        meta_t[:, g : g + 1].to_broadcast([128, 128]),
                        mybir.AluOpType.mult,
                    )

            # 4 groups share one PSUM bank; one batched ACT copy per bank;
            # out-DMA per 3 banks (fewer HWDGE configs on the SP queue)
            oap = out.ap()
            nb = (G + 3) // 4
            pend_lo = 0
            for bk in range(nb):
                glo = bk * 4
                ghi = min(glo + 4, G)
                span = ghi - glo
                ps = psp.tile([128, 512], f32, tag="ps")
                for j in range(span):
                    g = glo + j
                    nc.tensor.matmul(
                        ps[:, j * 128 : (j + 1) * 128],
                        tok[:, g, :],
                        oh[:, g, :],
                        start=True,
                        stop=True,
                    )
                nc.scalar.activation(
                    osb[:, glo * 128 : ghi * 128],
                    ps[:, 0 : span * 128],
                    mybir.ActivationFunctionType.Copy,
                )
                if bk % 3 == 2 or bk == nb - 1:
                    lo, hi = pend_lo * 512, glo * 128 + span * 128
                    nc.sync.dma_start(oap[:, lo:hi], osb[:, lo:hi])
                    pend_lo = bk + 1

    nc.compile()
    return nc


def _wrap16(a):
    """[M] -> [16, M/16] wrap (token j at [j%16, j//16]) replicated to 128 parts."""
    m = a.shape[0]
    w = a.reshape(m // 16, 16).T  # [16, M/16]
    return np.tile(w, (8, 1)).copy()  # [128, M/16]


def _preprocess(score, indices, dispatch, weight):
    sc = np.ascontiguousarray(np.asarray(score, dtype=np.float32)).reshape(B, EC)
    ix = np.asarray(indices).astype(np.int64, copy=False).reshape(B, EC)
    dp = np.asarray(dispatch).astype(np.int64, copy=False).reshape(B, EC)

    flat_core = (ix // WIN).ravel()
    flat_b = np.repeat(np.arange(B, dtype=np.int64), EC)
    flat_ixr = (ix % WIN).ravel()
    flat_dest = (flat_b * N + dp.ravel()).astype(np.int64)  # full output row
    flat_sc = sc.ravel()

    counts = np.bincount(flat_core, minlength=NCORES)
    maxtok = int(counts.max())
    G = (maxtok + 127) // 128
    TOT = G * 128

    # stable sort by (core, dest): dest-sorted within each core maximizes
    # rank compression within 128-token blocks
    key = flat_core * (B * N) + flat_dest
    order = np.argsort(key, kind="stable")
    s_core = flat_core[order]
    s_ixr = flat_ixr[order]
    s_dest = flat_dest[order]
    s_sc = flat_sc[order]

    starts = np.zeros(NCORES + 1, np.int64)
    np.add.at(starts, s_core + 1, 1)
    starts = np.cumsum(starts)
    within = np.arange(len(s_core)) - starts[s_core]

    gidx_all = np.zeros((NCORES, TOT), np.int16)
    score_all = np.zeros((NCORES, TOT), np.float32)
    dest_all = np.full((NCORES, TOT), -1, np.int64)
    gidx_all[s_core, within] = s_ixr.astype(np.int16)
    score_all[s_core, within] = s_sc
    dest_all[s_core, within] = s_dest

    # per block (128 consecutive tokens): rank-compress dests
    destrel_all = np.full((NCORES, TOT), -1.0, np.float32)
    rowmaps = np.full((NCORES, G, 128), -1, np.int64)
    for c in range(NCORES):
        d = dest_all[c].reshape(G, 128)
        for g in range(G):
            blk = d[g]
            valid = blk >= 0
            if not valid.any():
                continue
            uniq, inv = np.unique(blk[valid], return_inverse=True)
            destrel_all[c, g * 128 : (g + 1) * 128][valid] = inv.astype(np.float32)
            rowmaps[c, g, : len(uniq)] = uniq

    weight_bf = np.asarray(weight, dtype=np.float32).astype(BF16)
    iota = np.tile(np.arange(128, dtype=np.float32), (128, 1)).astype(BF16)

    in_maps = []
    for c in range(NCORES):
        sc_s = score_all[c].reshape(G, 128).T  # [128, G]
        dr_s = destrel_all[c].reshape(G, 128).T
        meta = np.ascontiguousarray(
            np.concatenate(
                [sc_s.astype(BF16), dr_s.astype(BF16), iota], axis=1
            )
        )
        in_maps.append(
            {
                "weight": np.ascontiguousarray(weight_bf[c * WIN : (c + 1) * WIN]),
                "gidx": _wrap16(gidx_all[c]),
                "meta": meta,
            }
        )
    return G, in_maps, rowmaps


def kernel(score, indices, dispatch, n, weight):
    global LAST_RESULTS
    from concourse import bass_utils

    assert int(np.asarray(n)) == N
    G, in_maps, rowmaps = _preprocess(score, indices, dispatch, weight)

    trace = _cache.pop("_trace_next", False)
    dummy = _cache.get("_flag_dummy", True)
    bigtt = _cache.get("_flag_bigtt", True)
    key = (G, trace, dummy, bigtt)
    if key not in _cache:
        _cache[key] = _build(G, dummy=dummy, bigtt=bigtt)
    nc = _cache[key]
    res = bass_utils.run_bass_kernel_spmd(
        nc, in_maps, core_ids=list(range(NCORES)), trace=trace
    )
    LAST_RESULTS = res

    out_full = np.zeros((B * N, D), np.float32)
    for c in range(NCORES):
        ot = res.results[c]["out"].astype(np.float32)  # [128, TOT]
        rm = rowmaps[c].reshape(-1)
        valid = rm >= 0
        np.add.at(out_full, rm[valid], ot[:, valid].T)
    return out_full.reshape(B, N, D)


# revision 11
# speedup vs baseline: 1.0577x; 1.0577x over previous
"""Trainium2 Bass kernel for scatter_memory problem nn_Memory_value_57475252355404.

out[b, dispatch[b,e,c], :] += weight[indices[b,e,c], :] * score[b,e,c]

Strategy (8 cores, SPMD single program, ONE launch):
  - Shard the TABLE row-wise: core k owns rows [k*32768, (k+1)*32768) and
    receives ONLY that 8MB bf16 slice as its per-core "weight" input, so
    the single SPMD program always gathers from window [0, 32768) and an
    int16 idx covers it exactly. Tokens are routed to cores by idx>>15.
  - Gather via SWDGE dma_gather (mlp ucode), ONE call per SWDGE queue
    (4 calls, descending sizes): no second wave per queue, so no
    ring-drain blocking. num_idxs is a compile-time constant (no cnt
    registers), pad slots point at row 0 with score 0 (no memset).
  - A dummy 128-idx gather from a memset idx tile is issued first so the
    one-time ~6us ucode IRAM load overlaps the input DMAs.
  - Scatter-add via per-block one-hot bf16 matmuls: block g = 128
    dest-sorted tokens; each distinct dest row in a block gets a rank
    slot; onehot[t, g*128+r] = (iota[r] == destrel[t,g]) * score[t,g],
    built with TWO whole-tile DVE tensor_tensor ops using broadcast APs;
    the PE computes psum[d, r] = sum_t tok[t, d] * onehot[t, r], 4 groups
    per PSUM bank; ACT copies each bank to a bf16 buffer; out-DMA per
    3 banks.
  - Host: rank slots -> physical rows (np.add.at in f32) over the full
    [B*N, D] output (cores may hit any row).
"""

import sys

sys.path.insert(0, "/opt/trn_rl_repo")

import numpy as np
import ml_dtypes

BF16 = ml_dtypes.bfloat16

B, E, C = 4, 16, 512
EC = E * C
V, D = 262144, 128
N = 4096
NCORES = 8
WIN = V // NCORES  # 32768 rows per core window
NQ = 4  # SWDGE queues / gather calls

_cache = {}
LAST_RESULTS = None  # BassKernelResults of the most recent run (for test.py)


MAXG_CALL = 8  # SWDGE ring holds 1024 descriptors -> at most 8*128 idxs/call


def _plan_calls(G):
    """Split G groups into gather calls of <=MAXG_CALL groups.

    Returns list of (queue, g_start, g_len) in EMISSION order; g ranges are
    assigned in emission order so the PE's in-order matmul stream matches
    chunk arrival order. Structure: a tiny "starter" call on q0 pays the
    one-time ucode IRAM load (~5us) and gets data flowing early; then one
    big call on each of q1..q3; then the rest back on q0 (its gen waits
    only for the tiny starter to drain).
    """
    assert G <= 4 * MAXG_CALL + 1, G
    starter = max(1, G - 4 * MAXG_CALL)
    rest = G - starter
    loads = []
    for i in range(4):
        q = (1 + i) % NQ  # q1, q2, q3, then q0 again
        share = min(MAXG_CALL, (rest + 3 - i) // (4 - i))
        loads.append((q, share))
        rest -= share
    assert rest == 0
    calls = [(0, 0, starter)]
    g = starter
    for q, share in loads:
        if share > 0:
            calls.append((q, g, share))
            g += share
    return calls


def _build(G, dummy=False, bigtt=True):
    from concourse import bacc, tile, mybir, library_config

    f32 = mybir.dt.float32
    bf16 = mybir.dt.bfloat16
    i16 = mybir.dt.int16

    TOT = G * 128
    calls = _plan_calls(G)

    nc = bacc.Bacc(
        "TRN2",
        target_bir_lowering=False,
        debug=False,
        num_devices=NCORES,
        num_swdge_queues=NQ,
    )
    w = nc.dram_tensor("weight", [WIN, D], bf16, kind="ExternalInput")
    gi = nc.dram_tensor("gidx", [128, TOT // 16], i16, kind="ExternalInput")
    # meta = score_s [128, G] | destrel [128, G] | iota [128, 128], all bf16
    meta = nc.dram_tensor("meta", [128, 2 * G + 128], bf16, kind="ExternalInput")
    out = nc.dram_tensor("out", [128, TOT], bf16, kind="ExternalOutput")

    with tile.TileContext(nc) as tc:
        with tc.tile_pool(name="p", bufs=1) as pool, \
             tc.tile_pool(name="ps", bufs=8, space="PSUM") as psp:
            # start the gpsimd ucode library load immediately; the dummy
            # gather right after pays the one-time IRAM load (~6us) while
            # the input DMAs are still in flight
            nc.gpsimd.load_library(library_config.mlp)
            wap = w.ap()
            if dummy:
                dum_i = pool.tile([128, 8], i16)
                nc.vector.memset(dum_i[:], 0)
                dum_o = pool.tile([128, 1, D], bf16)
                nc.gpsimd.dma_gather(
                    dum_o[:], wap, dum_i[:], 128, 128, D, queue_num=0
                )

            gi_t = pool.tile([128, TOT // 16], i16)
            nc.sync.dma_start(gi_t[:], gi.ap())
            meta_t = pool.tile([128, 2 * G + 128], bf16)
            nc.sync.dma_start(meta_t[:], meta.ap())

            tok = pool.tile([128, G, D], bf16)
            oh = pool.tile([128, G, 128], bf16)
            osb = pool.tile([128, TOT], bf16)

            # wave-1 calls (one per queue) then small wave-2 remainders
            for q, g0, glen in calls:
                cap = glen * 128
                off = g0 * 128
                nc.gpsimd.dma_gather(
                    tok[:, g0 : g0 + glen, :],
                    wap,
                    gi_t[:, off // 16 : (off + cap) // 16],
                    cap,
                    cap,
                    D,
                    queue_num=q,
                )

            # onehot[t, g, r] = (iota[r] == destrel[t,g]) * score[t,g]
            if bigtt:
                io_bc = meta_t[:, 2 * G : 2 * G + 128][:, None, :].to_broadcast(
                    [128, G, 128]
                )
                dr_bc = meta_t[:, G : 2 * G, None].to_broadcast([128, G, 128])
                sc_bc = meta_t[:, 0:G, None].to_broadcast([128, G, 128])
                nc.vector.tensor_tensor(oh[:], io_bc, dr_bc, mybir.AluOpType.is_equal)
                nc.vector.tensor_tensor(oh[:], oh[:], sc_bc, mybir.AluOpType.mult)
            else:
                io_t = meta_t[:, 2 * G : 2 * G + 128]
                for g in range(G):
                    nc.vector.tensor_tensor(
                        oh[:, g, :],
                        io_t,
                        meta_t[:, G + g : G + g + 1].to_broadcast([128, 128]),
                        mybir.AluOpType.is_equal,
                    )
                    nc.vector.tensor_tensor(
                        oh[:, g, :],
                        oh[:, g, :],
                        meta_t[:, g : g + 1].to_broadcast([128, 128]),
                        mybir.AluOpType.mult,
                    )

            # 4 groups share one PSUM bank; one batched ACT copy per bank;
            # out-DMA per 3 banks (fewer HWDGE configs on the SP queue)
            oap = out.ap()
            nb = (G + 3) // 4
            pend_lo = 0
            for bk in range(nb):
                glo = bk * 4
                ghi = min(glo + 4, G)
                span = ghi - glo
                ps = psp.tile([128, 512], f32, tag="ps")
                for j in range(span):
                    g = glo + j
                    nc.tensor.matmul(
                        ps[:, j * 128 : (j + 1) * 128],
                        tok[:, g, :],
                        oh[:, g, :],
                        start=True,
                        stop=True,
                    )
                nc.scalar.activation(
                    osb[:, glo * 128 : ghi * 128],
                    ps[:, 0 : span * 128],
                    mybir.ActivationFunctionType.Copy,
                )
                if bk % 3 == 2 or bk == nb - 1:
                    lo, hi = pend_lo * 512, glo * 128 + span * 128
                    nc.sync.dma_start(oap[:, lo:hi], osb[:, lo:hi])
                    pend_lo = bk + 1

    nc.compile()
    return nc


def _wrap16(a):
    """[M] -> [16, M/16] wrap (token j at [j%16, j//16]) replicated to 128 parts."""
    m = a.shape[0]
    w = a.reshape(m // 16, 16).T  # [16, M/16]
    return np.tile(w, (8, 1)).copy()  # [128, M/16]


def _preprocess(score, indices, dispatch, weight):
    sc = np.ascontiguousarray(np.asarray(score, dtype=np.float32)).reshape(B, EC)
    ix = np.asarray(indices).astype(np.int64, copy=False).reshape(B, EC)
    dp = np.asarray(dispatch).astype(np.int64, copy=False).reshape(B, EC)

    flat_core = (ix // WIN).ravel()
    flat_b = np.repeat(np.arange(B, dtype=np.int64), EC)
    flat_ixr = (ix % WIN).ravel()
    flat_dest = (flat_b * N + dp.ravel()).astype(np.int64)  # full output row
    flat_sc = sc.ravel()

    counts = np.bincount(flat_core, minlength=NCORES)
    maxtok = int(counts.max())
    G = (maxtok + 127) // 128
    TOT = G * 128

    # stable sort by (core, dest): dest-sorted within each core maximizes
    # rank compression within 128-token blocks
    key = flat_core * (B * N) + flat_dest
    order = np.argsort(key, kind="stable")
    s_core = flat_core[order]
    s_ixr = flat_ixr[order]
    s_dest = flat_dest[order]
    s_sc = flat_sc[order]

    starts = np.zeros(NCORES + 1, np.int64)
    np.add.at(starts, s_core + 1, 1)
    starts = np.cumsum(starts)
    within = np.arange(len(s_core)) - starts[s_core]

    gidx_all = np.zeros((NCORES, TOT), np.int16)
    score_all = np.zeros((NCORES, TOT), np.float32)
    dest_all = np.full((NCORES, TOT), -1, np.int64)
    gidx_all[s_core, within] = s_ixr.astype(np.int16)
    score_all[s_core, within] = s_sc
    dest_all[s_core, within] = s_dest

    # per block (128 consecutive tokens): rank-compress dests
    destrel_all = np.full((NCORES, TOT), -1.0, np.float32)
    rowmaps = np.full((NCORES, G, 128), -1, np.int64)
    for c in range(NCORES):
        d = dest_all[c].reshape(G, 128)
        for g in range(G):
            blk = d[g]
            valid = blk >= 0
            if not valid.any():
                continue
            uniq, inv = np.unique(blk[valid], return_inverse=True)
            destrel_all[c, g * 128 : (g + 1) * 128][valid] = inv.astype(np.float32)
            rowmaps[c, g, : len(uniq)] = uniq

    weight_bf = np.asarray(weight, dtype=np.float32).astype(BF16)
    iota = np.tile(np.arange(128, dtype=np.float32), (128, 1)).astype(BF16)

    in_maps = []
    for c in range(NCORES):
        sc_s = score_all[c].reshape(G, 128).T  # [128, G]
        dr_s = destrel_all[c].reshape(G, 128).T
        meta = np.ascontiguousarray(
            np.concatenate(
                [sc_s.astype(BF16), dr_s.astype(BF16), iota], axis=1
            )
        )
        in_maps.append(
            {
                "weight": np.ascontiguousarray(weight_bf[c * WIN : (c + 1) * WIN]),
                "gidx": _wrap16(gidx_all[c]),
                "meta": meta,
            }
        )
    return G, in_maps, rowmaps


def kernel(score, indices, dispatch, n, weight):
    global LAST_RESULTS
    from concourse import bass_utils

    assert int(np.asarray(n)) == N
    G, in_maps, rowmaps = _preprocess(score, indices, dispatch, weight)

    trace = _cache.pop("_trace_next", False)
    dummy = _cache.get("_flag_dummy", False)
    bigtt = _cache.get("_flag_bigtt", True)
    key = (G, trace, dummy, bigtt)
    if key not in _cache:
        _cache[key] = _build(G, dummy=dummy, bigtt=bigtt)
    nc = _cache[key]
    res = bass_utils.run_bass_kernel_spmd(
        nc, in_maps, core_ids=list(range(NCORES)), trace=trace
    )
    LAST_RESULTS = res

    out_full = np.zeros((B * N, D), np.float32)
    for c in range(NCORES):
        ot = res.results[c]["out"].astype(np.float32)  # [128, TOT]
        rm = rowmaps[c].reshape(-1)
        valid = rm >= 0
        np.add.at(out_full, rm[valid], ot[:, valid].T)
    return out_full.reshape(B, N, D)
